# revision 5
# baseline (speedup 1.0000x reference)
"""Trainium2 Bass kernel v2 for nn_CrossAttention (B=2, L=1024, S=2048, DIM=1024, H=16).

Core c handles batch b = c//4, head-group g = c%4 (4 heads, M=256 channels).

Design (vs v1 baseline at ~193us):
 - One continuous 64-step (lch, pair, st) softmax stream keeps the ACT engine
   (exp, the ~71us bottleneck) saturated; projections / V-proj / AV / Wo are
   attached to steps at fixed quotas so the in-order PE queue never blocks
   the next ST pair for long.
 - Host pre-tiles all inputs so every DMA is a contiguous multi-KB/partition
   read, split across the two HWDGE rings (sync + scalar) in arrival-priority
   order.
 - Softmax normalization: reciprocal_approx_fast (DVE, ~5x faster than
   reciprocal) + gpsimd partition_broadcast (replaces the K=1 broadcast
   matmul: no PSUM bank, no PE queue stall).
 - PSUM budget (8 banks): tag "st" 2 bufs x 2 banks (ST psum, reused by Wo
   psum post-stream) + tag "o" 4 bufs x 1 bank (proj accumulators, V-proj,
   AV accumulators).
"""

import sys

if "/opt/trn_rl_repo" not in sys.path:
    sys.path.insert(0, "/opt/trn_rl_repo")

import numpy as np

B, L, S, C = 2, 1024, 2048, 1024
NH, D = 16, 64
HPC = 4
M = HPC * D             # 256
SCALE = D ** -0.5
P = 128
NCORES = 8
CK = C // P             # 8
NST = S // P            # 16
LCH = 512
NLCH = L // LCH         # 2

_cache = {}


def _build(debug=False):
    import concourse.tile as tile
    from concourse import mybir, bacc

    f32 = mybir.dt.float32
    f32r = mybir.dt.float32r
    bf16 = mybir.dt.bfloat16
    Exp = mybir.ActivationFunctionType.Exp

    nc = bacc.Bacc("TRN2", target_bir_lowering=False, debug=False)

    # host-pretiled inputs (see _make_in_maps)
    xq = nc.dram_tensor("xq", [CK, P, L], bf16, kind="ExternalInput")
    xk = nc.dram_tensor("xk", [CK, P, S], bf16, kind="ExternalInput")
    xv = nc.dram_tensor("xv", [CK, P, S], bf16, kind="ExternalInput")
    wq = nc.dram_tensor("wq", [2, P, CK, P], bf16, kind="ExternalInput")
    wk = nc.dram_tensor("wk", [2, P, CK, P], bf16, kind="ExternalInput")
    wv = nc.dram_tensor("wv", [P, CK, M], bf16, kind="ExternalInput")
    wo = nc.dram_tensor("wo", [P, 2, C], bf16, kind="ExternalInput")
    outp = nc.dram_tensor("outp", [L, C], f32, kind="ExternalOutput")
    if debug:
        dbg_qt = nc.dram_tensor("dbg_qt", [P, 2, L], bf16, kind="ExternalOutput")
        dbg_kt = nc.dram_tensor("dbg_kt", [P, 2, S], bf16, kind="ExternalOutput")
        dbg_vones = nc.dram_tensor("dbg_vones", [P, NST, HPC, D + 1], bf16, kind="ExternalOutput")
        dbg_xgt = nc.dram_tensor("dbg_xgt", [P, 2, L], bf16, kind="ExternalOutput")
        dbg_pt = nc.dram_tensor("dbg_pt", [P, 2, LCH], bf16, kind="ExternalOutput")
        dbg_sums = nc.dram_tensor("dbg_sums", [2, 1, LCH], f32, kind="ExternalOutput")
        dbg_rc = nc.dram_tensor("dbg_rc", [2, 1, LCH], f32, kind="ExternalOutput")
        dbg_bc = nc.dram_tensor("dbg_bc", [2, D, LCH], f32, kind="ExternalOutput")
        dbg_o = nc.dram_tensor("dbg_o", [2, D, LCH], f32, kind="ExternalOutput")

    with tile.TileContext(nc) as tc:
        with tc.tile_pool(name="singles", bufs=1) as singles, \
             tc.tile_pool(name="pts", bufs=35 - (1 if debug else 0)) as pts, \
             tc.tile_pool(name="small", bufs=2) as small, \
             tc.tile_pool(name="obs", bufs=2) as obs, \
             tc.tile_pool(name="ps_st", bufs=2, space="PSUM") as pst, \
             tc.tile_pool(name="ps_o", bufs=4, space="PSUM") as po:

            # ---------------- persistent SBUF ----------------
            wq_sb = singles.tile([P, 2, CK, P], bf16, tag="wq")
            wk_sb = singles.tile([P, 2, CK, P], bf16, tag="wk")
            wv_sb = singles.tile([P, CK, M], bf16, tag="wv")
            wo_sb = singles.tile([P, 2, C], bf16, tag="wo")
            xq_sb = singles.tile([P, CK, L], bf16, tag="xq")
            xk_sb = singles.tile([P, CK, S], bf16, tag="xk")
            xv_sb = singles.tile([P, CK, S], bf16, tag="xv")
            qt_sb = singles.tile([P, 2, L], bf16, tag="qt")
            kt_sb = singles.tile([P, 2, S], bf16, tag="kt")
            vones = singles.tile([P, NST, HPC, D + 1], bf16, tag="vones")
            xgt_sb = singles.tile([P, 2, L], bf16, tag="xgt")

            # DMA: ~23GB/s per engine, ~640ns per HWDGE trigger. Critical
            # prefix split across rings: QT-path on sync, KT-path on scalar
            # (only 12 triggers ahead of the exps), V-path on the otherwise
            # idle gpsimd SWDGE ring.
            for h in range(2):
                nc.sync.dma_start(wq_sb[:, 0, 4 * h:4 * h + 4, :],
                                  wq[0, :, 4 * h:4 * h + 4, :])
            for ck in range(CK):
                nc.sync.dma_start(xq_sb[:, ck, 0:512], xq[ck, :, 0:512])
            for ck in range(CK):
                nc.sync.dma_start(xk_sb[:, ck, 512:1024], xk[ck, :, 512:1024])
            for h in range(2):
                nc.sync.dma_start(wq_sb[:, 1, 4 * h:4 * h + 4, :],
                                  wq[1, :, 4 * h:4 * h + 4, :])
            for ck in range(CK):
                nc.sync.dma_start(xk_sb[:, ck, 1024:1536], xk[ck, :, 1024:1536])
            for ck in range(CK):
                nc.sync.dma_start(xq_sb[:, ck, 512:1024], xq[ck, :, 512:1024])
            for ck in range(CK):
                nc.sync.dma_start(xk_sb[:, ck, 1536:2048], xk[ck, :, 1536:2048])
            # gpsimd SWDGE ring: KT critical prefix first (keeps the ACT
            # queue free of DMA triggers), then V-path + output weights
            for h in range(2):
                nc.gpsimd.dma_start(wk_sb[:, 0, 4 * h:4 * h + 4, :],
                                    wk[0, :, 4 * h:4 * h + 4, :])
            for ck in range(CK):
                nc.gpsimd.dma_start(xk_sb[:, ck, 0:512], xk[ck, :, 0:512])
            for h in range(2):
                nc.gpsimd.dma_start(wk_sb[:, 1, 4 * h:4 * h + 4, :],
                                    wk[1, :, 4 * h:4 * h + 4, :])
            nc.gpsimd.dma_start(wv_sb[:, 0:4, :], wv[:, 0:4, :])
            nc.gpsimd.dma_start(wv_sb[:, 4:8, :], wv[:, 4:8, :])
            for ck in range(CK):
                nc.gpsimd.dma_start(xv_sb[:, ck, 0:1024], xv[ck, :, 0:1024])
            for ck in range(CK):
                nc.gpsimd.dma_start(xv_sb[:, ck, 1024:2048], xv[ck, :, 1024:2048])
            nc.gpsimd.dma_start(wo_sb[:], wo[:])

            # ones column for the folded softmax-denominator matmul:
            # memset everything to 1.0; V casts later overwrite cols 0:D.
            nc.vector.memset(vones[:], 1.0)
            ones64 = singles.tile([1, D], f32, tag="ones64")
            nc.vector.memset(ones64[:], 1.0)

            # ---------------- helpers ----------------
            def proj_group(w_sb_of, x_of, dst, name):
                """One [128,512] projection accumulation over CK + cast."""
                ps = po.tile([P, 512], f32, tag="scratch", bufs=2, name=name)
                for ck in range(CK):
                    nc.tensor.matmul(ps[:], w_sb_of(ck), x_of(ck),
                                     start=(ck == 0), stop=(ck == CK - 1))
                nc.vector.tensor_copy(dst, ps[:])

            def qt_group(mt, lh):
                proj_group(
                    lambda ck: wq_sb[:, mt, ck, :],
                    lambda ck: xq_sb[:, ck, lh * 512:(lh + 1) * 512],
                    qt_sb[:, mt, lh * 512:(lh + 1) * 512],
                    f"qtps_{mt}_{lh}")

            def kt_group(mt, nch):
                proj_group(
                    lambda ck: wk_sb[:, mt, ck, :],
                    lambda ck: xk_sb[:, ck, nch * 512:(nch + 1) * 512],
                    kt_sb[:, mt, nch * 512:(nch + 1) * 512],
                    f"ktps_{mt}_{nch}")

            v_step = {}

            def v_group(st):
                """V proj for one s-tile (8 matmuls, N=256) + cast to vones."""
                vp = po.tile([P, M], f32, tag="scratch", bufs=2, name=f"vps_{st}")
                for ck in range(CK):
                    nc.tensor.matmul(
                        vp[:],
                        xv_sb[:, ck, st * P:(st + 1) * P],
                        wv_sb[:, ck, :],
                        start=(ck == 0), stop=(ck == CK - 1))
                nc.vector.tensor_copy(
                    vones[:, st, :, 0:D],
                    vp[:].rearrange("p (h d) -> p h d", h=HPC))

            def st_step(lch, pair, st):
                lsl = slice(lch * LCH, (lch + 1) * LCH)
                ssl = slice(st * P, (st + 1) * P)
                st_ps = pst.tile([P, 2, LCH], f32, tag="st",
                                 name=f"stps_{lch}_{pair}_{st}")
                nc.tensor.matmul(
                    st_ps[:, 0, :], kt_sb[0:D, pair, ssl], qt_sb[0:D, pair, lsl],
                    start=True, stop=True)
                nc.tensor.matmul(
                    st_ps[:, 1, :], kt_sb[D:P, pair, ssl], qt_sb[D:P, pair, lsl],
                    start=True, stop=True, tile_position=(64, 0))
                pt_t = pts.tile([P, 2, LCH], bf16, tag="pt",
                                name=f"pt_{lch}_{pair}_{st}")
                nc.scalar.activation(pt_t[:], st_ps[:], Exp, scale=SCALE)
                return pt_t

            def norm_phase1(lch, pair, o_ps):
                """4 partial reciprocals per hh (tracked DVE ops)."""
                rcps = []
                for hh in range(2):
                    for j in range(4):
                        rcp = small.tile([1, 128], f32, tag="rc2", bufs=16,
                                         name=f"rcp_{lch}_{pair}_{hh}_{j}")
                        with nc.allow_low_precision(reason="softmax denom"):
                            nc.vector.reciprocal(
                                rcp[:].bitcast(f32r),
                                o_ps[hh][D:D + 1, j * 128:(j + 1) * 128])
                        rcps.append(rcp)
                return rcps

            def norm_phase2(lch, pair, o_ps, rcps):
                """broadcast matmuls + final scale; issued ~2 steps after
                phase 1 so the PE queue never waits on the reciprocals."""
                lsl = slice(lch * LCH, (lch + 1) * LCH)
                for hh in range(2):
                    bc_ps = po.tile([D, LCH], f32, tag="scratch", bufs=2,
                                    name=f"bcps_{lch}_{pair}_{hh}")
                    for j in range(4):
                        nc.tensor.matmul(
                            bc_ps[:, j * 128:(j + 1) * 128],
                            ones64[:].bitcast(f32r),
                            rcps[hh * 4 + j][:].bitcast(f32r),
                            start=True, stop=True)
                    bc = small.tile([D, LCH], f32, tag="bc")
                    nc.vector.tensor_copy(bc[:], bc_ps[:])
                    nc.vector.tensor_mul(
                        xgt_sb[hh * D:(hh + 1) * D, pair, lsl],
                        o_ps[hh][0:D, :], bc[:])

            def wo_job(lt, nch):
                wo_ps = pst.tile([P, 2, 512], f32, tag="st",
                                 name=f"wops_{lt}_{nch}")
                for kt in range(2):
                    nc.tensor.matmul(
                        wo_ps[:, 0, :], xgt_sb[:, kt, lt * P:(lt + 1) * P],
                        wo_sb[:, kt, nch * 512:(nch + 1) * 512],
                        start=(kt == 0), stop=(kt == 1))
                ob = obs.tile([P, 512], f32, tag="ob")
                if (lt + nch) % 2 == 0:
                    nc.scalar.copy(ob[:], wo_ps[:, 0, :])
                else:
                    nc.vector.tensor_copy(ob[:], wo_ps[:, 0, :])
                nc.sync.dma_start(
                    outp[lt * P:(lt + 1) * P, nch * 512:(nch + 1) * 512], ob[:])

            # ---------------- pre-stream ----------------
            qt_group(0, 0)
            kt_group(0, 0)

            # step -> projection group; each lands after its DMA arrival
            # and >=2 steps before the first ST/AV that consumes it
            a_sched = {
                3: lambda: kt_group(0, 1),
                6: lambda: kt_group(0, 2),
                9: lambda: kt_group(1, 0),
                10: lambda: qt_group(1, 0),
                11: lambda: kt_group(0, 3),
                12: lambda: kt_group(1, 1),
                13: lambda: kt_group(1, 2),
                14: lambda: kt_group(1, 3),
                16: lambda: qt_group(0, 1),
                18: lambda: qt_group(1, 1),
            }

            # ---------------- the 64-step stream ----------------
            pt_store = {}
            o_tiles = {}

            def get_o(lch, pair):
                if (lch, pair) not in o_tiles:
                    o_tiles[(lch, pair)] = [
                        po.tile([D + 1, LCH], f32, tag="oacc", bufs=2,
                                name=f"ops_{lch}_{pair}_{hh}")
                        for hh in range(2)]
                return o_tiles[(lch, pair)]

            pending_norm = [None]   # (lch, pair, o_ps, rcps, age)

            def tick_pending():
                if pending_norm[0] is not None:
                    lch, pair, o_ps, rcps, age = pending_norm[0]
                    if age >= 5:
                        norm_phase2(lch, pair, o_ps, rcps)
                        pending_norm[0] = None
                    else:
                        pending_norm[0] = (lch, pair, o_ps, rcps, age + 1)

            def do_av(job):
                lch, pair, st, first, last = job
                o_ps = get_o(lch, pair)
                pt_t = pt_store.pop((lch, pair, st))
                for hh in range(2):
                    nc.tensor.matmul(
                        o_ps[hh][:, :], vones[:, st, pair * 2 + hh, :],
                        pt_t[:, hh, :],
                        start=first, stop=last)
                if last:
                    rcps = norm_phase1(lch, pair, o_ps)
                    pending_norm[0] = (lch, pair, o_ps, rcps, 0)
                    del o_tiles[(lch, pair)]

            ST_ORDER = list(range(NST))

            def av_jobs_of(lch):
                return [(lch, pair, st, i == 0, i == NST - 1)
                        for pair in range(2)
                        for i, st in enumerate(ST_ORDER)]

            v_jobs = list(range(NST))
            av0_jobs = av_jobs_of(0)
            av1_jobs = av_jobs_of(1)
            V_STEPS = [8, 10, 12, 14, 16, 17, 19, 20, 22, 23,
                       25, 26, 28, 29, 31, 32]

            for k in range(64):
                lch, pair, st = k // 32, (k // 16) % 2, k % 16
                # A: projections (issued BEFORE this step's ST so every
                # kt/qt region is written in program order before use)
                if k in a_sched:
                    a_sched.pop(k)()
                pt_store[(lch, pair, st)] = st_step(lch, pair, st)
                # B: V-projection groups at fixed steps
                if k in V_STEPS:
                    sv = v_jobs.pop(0)
                    v_group(sv)
                    v_step[sv] = k
                # C: AV, 2 jobs/step from step 33: lch0 first, then lch1
                # ready-paced; while a norm is pending, only same-pair jobs
                # may issue (the next pair would reclaim the o-slot early)
                tick_pending()
                if k >= 28:
                    budget = 2

                    def may(job):
                        if pending_norm[0] is None:
                            return True
                        return not job[3]   # job[3] == first (new pair)
                    while budget and av0_jobs and may(av0_jobs[0]):
                        do_av(av0_jobs.pop(0))
                        budget -= 1
                    while budget and av1_jobs and may(av1_jobs[0]):
                        i = 32 - len(av1_jobs)
                        if 32 + i <= k:
                            do_av(av1_jobs.pop(0))
                            budget -= 1
                        else:
                            break

            # ---------------- post-stream tail ----------------
            wo_tail = [(lt, nch) for lt in range(4) for nch in range(2)]
            while av1_jobs:
                tick_pending()
                if not may(av1_jobs[0]) if False else (
                        pending_norm[0] is not None and av1_jobs[0][3]):
                    # fill the wait with lch0 Wo jobs
                    if wo_tail:
                        wo_job(*wo_tail.pop(0))
                    else:
                        tick_pending()
                        if pending_norm[0] is not None:
                            norm_phase2(*pending_norm[0][:4])
                            pending_norm[0] = None
                    continue
                do_av(av1_jobs.pop(0))
            tick_pending()
            if pending_norm[0] is not None:
                norm_phase2(*pending_norm[0][:4])
                pending_norm[0] = None
            for lt, nch in wo_tail + [(lt, nch) for lt in range(4, 8)
                                      for nch in range(2)]:
                wo_job(lt, nch)

    nc.compile()
    return nc


def _get_nc(debug=False):
    key = ("nc", debug)
    if key not in _cache:
        _cache[key] = _build(debug=debug)
    return _cache[key]


def _make_in_maps(inputs):
    import ml_dtypes

    bf16 = ml_dtypes.bfloat16
    query = np.asarray(inputs["query"], dtype=np.float32)
    key = np.asarray(inputs["key"], dtype=np.float32)
    value = np.asarray(inputs["value"], dtype=np.float32)
    Wq = np.asarray(inputs["Wq"], dtype=np.float32)
    Wk = np.asarray(inputs["Wk"], dtype=np.float32)
    Wv = np.asarray(inputs["Wv"], dtype=np.float32)
    Wo = np.asarray(inputs["Wo"], dtype=np.float32)

    # activations: [ck, p, inner] tiles (contiguous views of the transposes)
    xq_t, xk_t, xv_t = [], [], []
    for b in range(B):
        qT = np.ascontiguousarray(query[b].T)            # [C, L]
        kT = np.ascontiguousarray(key[b].T)              # [C, S]
        vT = np.ascontiguousarray(value[b].T)            # [C, S]
        xq_t.append(qT.reshape(CK, P, L).astype(bf16))
        xk_t.append(kT.reshape(CK, P, S).astype(bf16))
        xv_t.append(vT.reshape(CK, P, S).astype(bf16))

    wq_s, wk_s, wv_s, wo_s = [], [], [], []
    for g in range(4):
        for W, out, mtsplit in ((Wq, wq_s, True), (Wk, wk_s, True),
                                (Wv, wv_s, False)):
            w = W[:, g * M:(g + 1) * M]                  # [C, M]
            t = w.reshape(CK, P, M).transpose(1, 0, 2)   # [p, ck, m]
            if mtsplit:
                t = t.reshape(P, CK, 2, P).transpose(2, 0, 1, 3)
            out.append(np.ascontiguousarray(t).astype(bf16))
        w = Wo[g * M:(g + 1) * M, :]                     # [M, C]
        wo_s.append(np.ascontiguousarray(
            w.reshape(2, P, C).transpose(1, 0, 2)).astype(bf16))

    in_maps = []
    for core in range(NCORES):
        b, g = core // 4, core % 4
        in_maps.append({
            "xq": xq_t[b], "xk": xk_t[b], "xv": xv_t[b],
            "wq": wq_s[g], "wk": wk_s[g], "wv": wv_s[g], "wo": wo_s[g],
        })
    return in_maps


def kernel(query, key, value, Wq, Wk, Wv, Wo, bo):
    from concourse.bass_utils import run_bass_kernel_spmd

    nc = _get_nc()
    bo = np.asarray(bo, dtype=np.float32)
    in_maps = _make_in_maps(dict(query=query, key=key, value=value,
                                 Wq=Wq, Wk=Wk, Wv=Wv, Wo=Wo))

    res = run_bass_kernel_spmd(nc, in_maps, core_ids=list(range(NCORES)))

    out = np.zeros((B, L, C), dtype=np.float32)
    for core in range(NCORES):
        b = core // 4
        out[b] += res.results[core]["outp"]
    out += bo[None, None, :]
    return out
